# revision 3
# baseline (speedup 1.0000x reference)
"""Deformable-attention Trainium2 kernel v2 (8-core SPMD, key-major).

Sharding: core c handles batch b = c//4 and heads h0 = 2*(c%4), h0+1.
Each core computes its heads' QKV projection, KEY-MAJOR attention
(S^T tiles: 128 keys x 2048 queries), and a partial output projection;
the host sums the 4 partials per batch and adds b_out.

v2 redesign vs v1: no DMA/PE transposes at all. S^T is computed
directly key-major (stationary K^T, moving esc-scaled Q^T), exp on the
scalar engine (esc folded into Q so ACT scale is 1), window masking as
two TENSOR_MASK custom-DVE passes against host-broadcast ms/-me rows
with per-partition key-index scalars, and PV accumulates over key
tiles straight from the masked E^T tiles (start/stop PSUM group).

Host-side control path (pure numpy, ~0.5% of FLOPs): od projection,
window bounds, and the additive point-weight correction C (<=4 cells
per query), applied during PV evacuation as in v1.
"""

import os
import numpy as np

B, T, E, NH = 2, 2048, 512, 8
HD = E // NH  # 64
SCALE = float(HD) ** -0.5
NCORES = 8
KT_TILES = T // 128  # 16
ECH = E // 128  # 4

_cache = {}


# ---------------------------------------------------------------- host prep
def _host_control(x, w_qkv, b_qkv, w_od, b_od):
    w_eff = (w_qkv[:, :E] @ w_od).astype(np.float32)
    b_eff = (b_qkv[:E] @ w_od + b_od).astype(np.float32)
    od = (x.reshape(B * T, E).astype(np.float32) @ w_eff + b_eff).reshape(
        B, T, 2 * NH
    )
    offset = np.tanh(od[..., :NH]) * T
    duration = 1.0 / (1.0 + np.exp(-od[..., NH:])) * T
    qidx = np.arange(T, dtype=np.float32)[None, :, None]
    anchor = (qidx + offset).transpose(0, 2, 1).reshape(B * NH, T)
    duration = duration.transpose(0, 2, 1).reshape(B * NH, T)
    start = anchor - duration
    end = anchor + duration
    bl = np.floor(start)
    br = np.ceil(end)
    al = np.floor(anchor)
    ar = al + 1.0
    frac = anchor - al

    empty = (br < 0) | (bl > T - 1)
    ms = np.where(empty, 0.0, np.maximum(bl, 0.0)).astype(np.float32)
    me = np.where(empty, float(T), np.minimum(br, T - 1) + 1.0).astype(np.float32)
    esc = np.where(empty, 0.0, SCALE).astype(np.float32)

    BH = B * NH
    cells = np.zeros((BH, T, 4), np.int64)
    coefs = np.zeros((BH, T, 4), np.float32)
    raw = [(bl, bl - start), (br, end - br), (ar, frac), (al, 1.0 - frac)]
    for k, (cell, coef) in enumerate(raw):
        valid = (cell >= 0) & (cell <= T - 1) & ~empty
        cells[:, :, k] = np.where(valid, cell, 0).astype(np.int64)
        coefs[:, :, k] = np.where(valid, coef, 0.0)
    return ms, me, esc, cells, coefs


def _host_correction(x, w_qkv, b_qkv, cells, coefs):
    """C[bh, t, HD+1] = sum over distinct cells of
    (exp(SCALE*s*(1+csum)) - exp(SCALE*s)) * Vaug[cell]; col HD is the
    ones-column (sum-of-weights) part."""
    xf = x.reshape(B * T, E).astype(np.float32)
    qkv = xf @ w_qkv + b_qkv
    Q, K, V = qkv[:, :E], qkv[:, E : 2 * E], qkv[:, 2 * E :]

    def heads(t):
        return (
            t.reshape(B, T, NH, HD).transpose(0, 2, 1, 3).reshape(B * NH, T, HD)
        )

    Qh, Kh, Vh = heads(Q), heads(K), heads(V)
    BH = B * NH
    order = np.argsort(cells, axis=2, kind="stable")
    cs = np.take_along_axis(cells, order, 2)
    cf = np.take_along_axis(coefs, order, 2).astype(np.float64)
    for k in range(3, 0, -1):
        dup = cs[:, :, k] == cs[:, :, k - 1]
        cf[:, :, k - 1] += np.where(dup, cf[:, :, k], 0.0)
        cf[:, :, k] = np.where(dup, 0.0, cf[:, :, k])
    C = np.zeros((BH, T, HD + 1), np.float32)
    for bh in range(BH):
        Kg = Kh[bh][cs[bh].reshape(-1)].reshape(T, 4, HD)
        s = SCALE * np.einsum("td,tkd->tk", Qh[bh], Kg)
        active = cf[bh] != 0.0
        dw = np.where(
            active, np.exp(s * (1.0 + cf[bh])) - np.exp(s), 0.0
        ).astype(np.float32)
        Vg = Vh[bh][cs[bh].reshape(-1)].reshape(T, 4, HD)
        C[bh, :, :HD] = np.einsum("tk,tkd->td", dw, Vg)
        C[bh, :, HD] = dw.sum(axis=1)
    return C


def _prep_core_inputs(inputs, reps):
    x = np.asarray(inputs["x"], np.float32)
    w_qkv = np.asarray(inputs["w_qkv"], np.float32)
    b_qkv = np.asarray(inputs["b_qkv"], np.float32)
    w_od = np.asarray(inputs["w_od"], np.float32)
    b_od = np.asarray(inputs["b_od"], np.float32)
    w_out = np.asarray(inputs["w_out"], np.float32)

    ms, me, esc, cells, coefs = _host_control(x, w_qkv, b_qkv, w_od, b_od)
    C = _host_correction(x, w_qkv, b_qkv, cells, coefs)

    # per-partition absolute key index columns (+kabs, -kabs) per kt
    kp = (np.arange(KT_TILES) * 128)[None, :] + np.arange(128)[:, None]
    iotaKp = np.stack([kp, -kp], axis=1).astype(np.float32)  # (128, 2, 16)

    in_maps = []
    for c in range(NCORES):
        b = c // 4
        h0 = 2 * (c % 4)
        bhs = [b * NH + h0, b * NH + h0 + 1]
        xT = np.ascontiguousarray(x[b].T).astype(np.float16)  # (E, T)
        escB = np.concatenate(
            [np.broadcast_to(esc[bh][None, :], (HD, T)) for bh in bhs], axis=0
        ).astype(np.float16)  # (128, T)
        msB = np.stack([ms[bh] for bh in bhs], axis=0)  # (2, T)
        msB = np.broadcast_to(msB[None], (128, 2, T)).astype(np.float16)
        negmeB = np.stack([-me[bh] for bh in bhs], axis=0)
        negmeB = np.broadcast_to(negmeB[None], (128, 2, T)).astype(np.float16)
        m = {
            "xT": xT.reshape(ECH, 128, T),
            "wq": np.ascontiguousarray(w_qkv[:, h0 * HD : (h0 + 2) * HD])
            .astype(np.float16)
            .reshape(ECH, 128, 128),
            "wk": np.ascontiguousarray(
                w_qkv[:, E + h0 * HD : E + (h0 + 2) * HD]
            )
            .astype(np.float16)
            .reshape(ECH, 128, 128),
            "wv": np.ascontiguousarray(
                w_qkv[:, 2 * E + h0 * HD : 2 * E + (h0 + 2) * HD]
            )
            .astype(np.float16)
            .reshape(ECH, 128, 128),
            "bq": b_qkv[h0 * HD : (h0 + 2) * HD].astype(np.float16).reshape(1, 128),
            "bk": b_qkv[E + h0 * HD : E + (h0 + 2) * HD]
            .astype(np.float16)
            .reshape(1, 128),
            "bv": b_qkv[2 * E + h0 * HD : 2 * E + (h0 + 2) * HD]
            .astype(np.float16)
            .reshape(1, 128),
            "escB": np.ascontiguousarray(escB),
            "msB": np.ascontiguousarray(msB),
            "negmeB": np.ascontiguousarray(negmeB),
            "iotaKp": iotaKp,
            "corrT": np.stack([C[bh].T for bh in bhs], axis=1).astype(
                np.float16
            ),  # (65, 2, 2048)
            "wout2": np.concatenate(
                [w_out[(h0 + hh) * HD : (h0 + hh + 1) * HD] for hh in range(2)],
                axis=0,
            ).astype(np.float16),  # (128, 512)
            "reps": np.array([[reps]], np.int32),
        }
        in_maps.append(m)
    return in_maps


# ---------------------------------------------------------------- bass build
def _split_drain_waits_json(bir: bytes) -> bytes:
    """Workaround: this walrus build rejects instructions carrying more
    than one sync-wait command. Hoist excess waits onto inserted
    single-wait NoOps on the same engine directly before the
    instruction (same-engine program order makes this equivalent)."""
    import json

    m = json.loads(bir)
    limit = 1
    for f in m["functions"]:
        for bb in f["blocks"]:
            out = []
            for inst in bb["instructions"]:
                si = inst.get("sync_info")
                waits = (si.get("on_wait") or []) if si else []
                if len(waits) > limit:
                    for j, w in enumerate(waits[:-limit]):
                        pre = {
                            "engine": inst["engine"],
                            "ins": [],
                            "outs": [],
                            "name": f"{inst['name']}_w{j}",
                            "opcode": "NoOp",
                            "sync_info": {"on_update": [], "on_wait": [w]},
                        }
                        if "debug" in inst:
                            pre["debug"] = inst["debug"]
                        out.append(pre)
                    si["on_wait"] = waits[-limit:]
                out.append(inst)
            bb["instructions"] = out
    return json.dumps(m).encode()


def _build_nc(loop=True):
    import contextlib

    import concourse.bass as bass
    import concourse.tile as tile
    from concourse import mybir

    f16, f32, i32 = mybir.dt.float16, mybir.dt.float32, mybir.dt.int32
    nc = bass.Bass()
    d_xT = nc.dram_tensor("xT", (ECH, 128, T), f16, kind="ExternalInput")
    d_wq = nc.dram_tensor("wq", (ECH, 128, 128), f16, kind="ExternalInput")
    d_wk = nc.dram_tensor("wk", (ECH, 128, 128), f16, kind="ExternalInput")
    d_wv = nc.dram_tensor("wv", (ECH, 128, 128), f16, kind="ExternalInput")
    d_bq = nc.dram_tensor("bq", (1, 128), f16, kind="ExternalInput")
    d_bk = nc.dram_tensor("bk", (1, 128), f16, kind="ExternalInput")
    d_bv = nc.dram_tensor("bv", (1, 128), f16, kind="ExternalInput")
    d_escB = nc.dram_tensor("escB", (128, T), f16, kind="ExternalInput")
    d_msB = nc.dram_tensor("msB", (128, 2, T), f16, kind="ExternalInput")
    d_negmeB = nc.dram_tensor("negmeB", (128, 2, T), f16, kind="ExternalInput")
    d_iotaKp = nc.dram_tensor("iotaKp", (128, 2, KT_TILES), f32, kind="ExternalInput")
    d_corr = nc.dram_tensor("corrT", (HD + 1, 2, T), f16, kind="ExternalInput")
    d_wout2 = nc.dram_tensor("wout2", (128, 512), f16, kind="ExternalInput")
    d_reps = nc.dram_tensor("reps", (1, 1), i32, kind="ExternalInput")
    d_y = nc.dram_tensor("y", (T, E), f32, kind="ExternalOutput")

    with tile.TileContext(nc) as tc, contextlib.ExitStack() as stk:
        consts = stk.enter_context(tc.tile_pool(name="consts", bufs=2))
        qk = stk.enter_context(tc.tile_pool(name="qk", bufs=1))
        epool = stk.enter_context(tc.tile_pool(name="epool", bufs=3))
        small = stk.enter_context(tc.tile_pool(name="small", bufs=1))
        spool = stk.enter_context(tc.tile_pool(name="spool", bufs=3, space="PSUM"))
        pvpool = stk.enter_context(
            tc.tile_pool(name="pvpool", bufs=1, space="PSUM")
        )

        reps_sb = consts.tile([1, 1], i32)
        nc.sync.dma_start(reps_sb[:], d_reps[:])
        if loop:
            reps_val = nc.values_load(
                reps_sb[:],
                min_val=1,
                max_val=100000,
                skip_runtime_bounds_check=True,
            )
            loop_cm = tc.For_i(0, reps_val)
        else:
            import contextlib as _cl

            loop_cm = _cl.nullcontext()
        with loop_cm:
            # ---- load inputs
            xT = consts.tile([128, ECH, T], f16)
            nc.sync.dma_start(xT[:], d_xT.rearrange("c p t -> p c t"))
            wq = consts.tile([128, ECH, 128], f16)
            wk = consts.tile([128, ECH, 128], f16)
            wv = consts.tile([128, ECH, 128], f16)
            nc.sync.dma_start(wq[:], d_wq.rearrange("c p m -> p c m"))
            nc.sync.dma_start(wk[:], d_wk.rearrange("c p m -> p c m"))
            nc.sync.dma_start(wv[:], d_wv.rearrange("c p m -> p c m"))
            bq = consts.tile([1, 128], f16)
            bk = consts.tile([1, 128], f16)
            bv = consts.tile([1, 128], f16)
            nc.sync.dma_start(bq[:], d_bq[:])
            nc.sync.dma_start(bk[:], d_bk[:])
            nc.sync.dma_start(bv[:], d_bv[:])
            escB = consts.tile([128, T], f16)
            nc.sync.dma_start(escB[:], d_escB[:])
            msB = consts.tile([128, 2, T], f16)
            negmeB = consts.tile([128, 2, T], f16)
            nc.scalar.dma_start(msB[:], d_msB[:])
            nc.scalar.dma_start(negmeB[:], d_negmeB[:])
            iotaKp = consts.tile([128, 2, KT_TILES], f32)
            nc.sync.dma_start(iotaKp[:], d_iotaKp[:])
            corrT = consts.tile([HD + 1, 2, T], f16)
            nc.sync.dma_start(corrT[:], d_corr[:])
            wout2 = consts.tile([128, 512], f16)
            nc.sync.dma_start(wout2[:], d_wout2[:])
            ones_row = consts.tile([1, 512], f16)
            nc.vector.memset(ones_row[:], 1.0)
            ones32 = consts.tile([1, HD], f32)
            nc.vector.memset(ones32[:], 1.0)

            # ---- QKV projection (both heads at once; chan-major Q^T/K^T)
            # QT holds esc-prescaled Q^T so S^T = esc_q * (Q K^T)[q,k].
            QT = qk.tile([128, T], f16)
            KT = qk.tile([128, T], f16)
            for t4 in range(T // 512):
                sl = slice(t4 * 512, (t4 + 1) * 512)
                psq = spool.tile([128, 512], f32, tag="s")
                for ec in range(ECH):
                    nc.tensor.matmul(
                        psq[:], wq[:, ec, :], xT[:, ec, sl],
                        start=(ec == 0), stop=False,
                    )
                nc.tensor.matmul(psq[:], bq[:], ones_row[:], start=False, stop=True)
                nc.vector.tensor_mul(QT[:, sl], psq[:], escB[:, sl])
                psk = spool.tile([128, 512], f32, tag="s")
                for ec in range(ECH):
                    nc.tensor.matmul(
                        psk[:], wk[:, ec, :], xT[:, ec, sl],
                        start=(ec == 0), stop=False,
                    )
                nc.tensor.matmul(psk[:], bk[:], ones_row[:], start=False, stop=True)
                nc.vector.tensor_copy(KT[:, sl], psk[:])
            # V token-major, ones-augmented: vaug[:, kt, 0:65 | 65:130]
            vaug = qk.tile([128, KT_TILES, 2 * (HD + 1)], f16)
            nc.vector.memset(vaug[:, :, HD : HD + 1], 1.0)
            nc.vector.memset(vaug[:, :, 2 * HD + 1 : 2 * HD + 2], 1.0)
            for kt in range(KT_TILES):
                psv = spool.tile([128, 512], f32, tag="s")
                tsl = slice(kt * 128, (kt + 1) * 128)
                for ec in range(ECH):
                    nc.tensor.matmul(
                        psv[:, 0:128], xT[:, ec, tsl], wv[:, ec, :],
                        start=(ec == 0), stop=False,
                    )
                nc.tensor.matmul(
                    psv[:, 0:128], ones_row[:, 0:128], bv[:],
                    start=False, stop=True,
                )
                nc.vector.tensor_copy(vaug[:, kt, 0:HD], psv[:, 0:HD])
                nc.vector.tensor_copy(
                    vaug[:, kt, HD + 1 : 2 * HD + 1], psv[:, HD : 2 * HD]
                )

            # ---- attention per head: key-major S^T -> exp -> mask -> PV
            onorm2 = qk.tile([128, T], f16)  # both heads' normalized PV
            for hh in range(2):
                hsl = slice(hh * HD, (hh + 1) * HD)
                vsl = slice(hh * (HD + 1), (hh + 1) * (HD + 1))
                pv = pvpool.tile([HD + 1, T], f32)
                pending = None  # (E tile, kt) awaiting PV emission
                for kt in range(KT_TILES):
                    ksl = slice(kt * 128, (kt + 1) * 128)
                    Ekt = epool.tile([128, T], f16, tag="e")
                    for qc in range(T // 512):
                        qsl = slice(qc * 512, (qc + 1) * 512)
                        sps = spool.tile([128, 512], f32, tag="s")
                        nc.tensor.matmul(
                            sps[:], KT[hsl, ksl], QT[hsl, qsl],
                            start=True, stop=True,
                        )
                        nc.scalar.activation(
                            Ekt[:, qsl], sps[:],
                            mybir.ActivationFunctionType.Exp,
                        )
                    mt = epool.tile([128, T], f16, tag="m1", name="mt")
                    nc.vector.tensor_scalar(
                        mt[:], msB[:, hh, :],
                        iotaKp[:, 0, kt : kt + 1], None,
                        mybir.AluOpType.is_le,
                    )
                    mt2 = epool.tile([128, T], f16, tag="m2", name="mt2")
                    nc.vector.tensor_scalar(
                        mt2[:], negmeB[:, hh, :],
                        iotaKp[:, 1, kt : kt + 1], None,
                        mybir.AluOpType.is_lt,
                    )
                    nc.vector.tensor_mul(Ekt[:], Ekt[:], mt[:])
                    nc.vector.tensor_mul(Ekt[:], Ekt[:], mt2[:])
                    if pending is not None:
                        pE, pkt = pending
                        for qc in range(T // 512):
                            qsl = slice(qc * 512, (qc + 1) * 512)
                            nc.tensor.matmul(
                                pv[:, qsl], vaug[:, pkt, vsl], pE[:, qsl],
                                start=(pkt == 0), stop=False,
                            )
                    pending = (Ekt, kt)
                pE, pkt = pending
                for qc in range(T // 512):
                    qsl = slice(qc * 512, (qc + 1) * 512)
                    nc.tensor.matmul(
                        pv[:, qsl], vaug[:, pkt, vsl], pE[:, qsl],
                        start=False, stop=True,
                    )
                # evacuate + host correction + normalize
                oaug = small.tile([HD + 1, T], f32, tag="oaug")
                nc.vector.tensor_add(oaug[:], pv[:], corrT[:, hh, :])
                sums = small.tile([1, T], f32, tag="sums")
                nc.sync.dma_start(sums[:], oaug[HD : HD + 1, :])
                recip = small.tile([1, T], f32, tag="recip")
                nc.vector.reciprocal_approx_fast(recip[:], sums[:])
                for qc in range(T // 512):
                    qsl = slice(qc * 512, (qc + 1) * 512)
                    bc = spool.tile([128, 512], f32, tag="s")
                    nc.tensor.matmul(
                        bc[0:HD, :], ones32[:], recip[:, qsl],
                        start=True, stop=True,
                    )
                    nc.vector.tensor_mul(
                        onorm2[hsl, qsl], oaug[0:HD, qsl], bc[0:HD, :]
                    )
            # ---- output projection (both heads in one K=128 contraction)
            for tt in range(T // 128):
                tsl = slice(tt * 128, (tt + 1) * 128)
                yp = spool.tile([128, 512], f32, tag="s")
                nc.tensor.matmul(
                    yp[:], onorm2[:, tsl], wout2[:], start=True, stop=True
                )
                y_sb = epool.tile([128, 512], f32, tag="ysb", name="y_sb")
                nc.scalar.copy(y_sb[:], yp[:])
                nc.sync.dma_start(d_y[tsl, :], y_sb[:])
    return nc


# ---------------------------------------------------------------- entry
def _get_nc():
    if "nc" not in _cache:
        from concourse import mybir

        nc = _build_nc()
        mybir.codegen_inst_isa_subclasses(nc)
        fixed = _split_drain_waits_json(nc.to_json_bytes())
        nc.to_json_bytes = lambda: fixed
        _cache["nc"] = nc
    return _cache["nc"]


def run_cores(inputs, reps=1):
    """Compile (cached) + run on 8 cores; returns list of per-core y."""
    from concourse.bass_utils import run_bass_kernel_spmd

    nc = _get_nc()
    in_maps = _prep_core_inputs(inputs, reps)
    res = run_bass_kernel_spmd(nc, in_maps, core_ids=list(range(NCORES)))
    return [r["y"] for r in res.results]


def kernel(**inputs):
    reps = int(os.environ.get("BASS_KERNEL_REPS", "1"))
    ys = run_cores(inputs, reps=reps)
    b_out = np.asarray(inputs["b_out"], np.float32)
    y = np.zeros((B, T, E), np.float32)
    for c in range(NCORES):
        y[c // 4] += ys[c]
    y += b_out[None, None, :]
    return y.astype(np.float32)


# revision 4
# speedup vs baseline: 4.1230x; 4.1230x over previous
"""Deformable-attention Trainium2 kernel v2 (8-core SPMD, key-major).

Sharding: core c handles batch b = c//4 and heads h0 = 2*(c%4), h0+1.
Each core computes its heads' QKV projection, KEY-MAJOR attention
(S^T tiles: 128 keys x 2048 queries), and a partial output projection;
the host sums the 4 partials per batch and adds b_out.

v2 redesign vs v1: no DMA/PE transposes at all. S^T is computed
directly key-major (stationary K^T, moving esc-scaled Q^T), exp on the
scalar engine (esc folded into Q so ACT scale is 1), window masking as
two TENSOR_MASK custom-DVE passes against host-broadcast ms/-me rows
with per-partition key-index scalars, and PV accumulates over key
tiles straight from the masked E^T tiles (start/stop PSUM group).

Host-side control path (pure numpy, ~0.5% of FLOPs): od projection,
window bounds, and the additive point-weight correction C (<=4 cells
per query), applied during PV evacuation as in v1.
"""

import os
import numpy as np

B, T, E, NH = 2, 2048, 512, 8
HD = E // NH  # 64
SCALE = float(HD) ** -0.5
NCORES = 8
KT_TILES = T // 128  # 16
ECH = E // 128  # 4

_cache = {}


# ---------------------------------------------------------------- host prep
def _host_control(x, w_qkv, b_qkv, w_od, b_od):
    w_eff = (w_qkv[:, :E] @ w_od).astype(np.float32)
    b_eff = (b_qkv[:E] @ w_od + b_od).astype(np.float32)
    od = (x.reshape(B * T, E).astype(np.float32) @ w_eff + b_eff).reshape(
        B, T, 2 * NH
    )
    offset = np.tanh(od[..., :NH]) * T
    duration = 1.0 / (1.0 + np.exp(-od[..., NH:])) * T
    qidx = np.arange(T, dtype=np.float32)[None, :, None]
    anchor = (qidx + offset).transpose(0, 2, 1).reshape(B * NH, T)
    duration = duration.transpose(0, 2, 1).reshape(B * NH, T)
    start = anchor - duration
    end = anchor + duration
    bl = np.floor(start)
    br = np.ceil(end)
    al = np.floor(anchor)
    ar = al + 1.0
    frac = anchor - al

    empty = (br < 0) | (bl > T - 1)
    ms = np.where(empty, 0.0, np.maximum(bl, 0.0)).astype(np.float32)
    me = np.where(empty, float(T), np.minimum(br, T - 1) + 1.0).astype(np.float32)
    esc = np.where(empty, 0.0, SCALE).astype(np.float32)

    BH = B * NH
    cells = np.zeros((BH, T, 4), np.int64)
    coefs = np.zeros((BH, T, 4), np.float32)
    raw = [(bl, bl - start), (br, end - br), (ar, frac), (al, 1.0 - frac)]
    for k, (cell, coef) in enumerate(raw):
        valid = (cell >= 0) & (cell <= T - 1) & ~empty
        cells[:, :, k] = np.where(valid, cell, 0).astype(np.int64)
        coefs[:, :, k] = np.where(valid, coef, 0.0)
    return ms, me, esc, cells, coefs


def _host_correction(x, w_qkv, b_qkv, cells, coefs):
    """C[bh, t, HD+1] = sum over distinct cells of
    (exp(SCALE*s*(1+csum)) - exp(SCALE*s)) * Vaug[cell]; col HD is the
    ones-column (sum-of-weights) part."""
    xf = x.reshape(B * T, E).astype(np.float32)
    qkv = xf @ w_qkv + b_qkv
    Q, K, V = qkv[:, :E], qkv[:, E : 2 * E], qkv[:, 2 * E :]

    def heads(t):
        return (
            t.reshape(B, T, NH, HD).transpose(0, 2, 1, 3).reshape(B * NH, T, HD)
        )

    Qh, Kh, Vh = heads(Q), heads(K), heads(V)
    BH = B * NH
    order = np.argsort(cells, axis=2, kind="stable")
    cs = np.take_along_axis(cells, order, 2)
    cf = np.take_along_axis(coefs, order, 2).astype(np.float64)
    for k in range(3, 0, -1):
        dup = cs[:, :, k] == cs[:, :, k - 1]
        cf[:, :, k - 1] += np.where(dup, cf[:, :, k], 0.0)
        cf[:, :, k] = np.where(dup, 0.0, cf[:, :, k])
    C = np.zeros((BH, T, HD + 1), np.float32)
    for bh in range(BH):
        Kg = Kh[bh][cs[bh].reshape(-1)].reshape(T, 4, HD)
        s = SCALE * np.einsum("td,tkd->tk", Qh[bh], Kg)
        active = cf[bh] != 0.0
        dw = np.where(
            active, np.exp(s * (1.0 + cf[bh])) - np.exp(s), 0.0
        ).astype(np.float32)
        Vg = Vh[bh][cs[bh].reshape(-1)].reshape(T, 4, HD)
        C[bh, :, :HD] = np.einsum("tk,tkd->td", dw, Vg)
        C[bh, :, HD] = dw.sum(axis=1)
    return C


def _prep_core_inputs(inputs, reps):
    x = np.asarray(inputs["x"], np.float32)
    w_qkv = np.asarray(inputs["w_qkv"], np.float32)
    b_qkv = np.asarray(inputs["b_qkv"], np.float32)
    w_od = np.asarray(inputs["w_od"], np.float32)
    b_od = np.asarray(inputs["b_od"], np.float32)
    w_out = np.asarray(inputs["w_out"], np.float32)

    ms, me, esc, cells, coefs = _host_control(x, w_qkv, b_qkv, w_od, b_od)
    C = _host_correction(x, w_qkv, b_qkv, cells, coefs)

    # per-partition absolute key index columns (+kabs, -kabs) per kt
    kp = (np.arange(KT_TILES) * 128)[None, :] + np.arange(128)[:, None]
    iotaKp = np.stack([kp, -kp], axis=1).astype(np.float32)  # (128, 2, 16)

    in_maps = []
    for c in range(NCORES):
        b = c // 4
        h0 = 2 * (c % 4)
        bhs = [b * NH + h0, b * NH + h0 + 1]
        xT = np.ascontiguousarray(x[b].T).astype(np.float16)  # (E, T)
        escB = np.concatenate(
            [np.broadcast_to(esc[bh][None, :], (HD, T)) for bh in bhs], axis=0
        ).astype(np.float16)  # (128, T)
        msB = np.stack([ms[bh] for bh in bhs], axis=0)  # (2, T)
        msB = np.broadcast_to(msB[None], (128, 2, T)).astype(np.float16)
        negmeB = np.stack([-me[bh] for bh in bhs], axis=0)
        negmeB = np.broadcast_to(negmeB[None], (128, 2, T)).astype(np.float16)
        m = {
            "xT": xT.reshape(ECH, 128, T),
            "wq": np.ascontiguousarray(w_qkv[:, h0 * HD : (h0 + 2) * HD])
            .astype(np.float16)
            .reshape(ECH, 128, 128),
            "wk": np.ascontiguousarray(
                w_qkv[:, E + h0 * HD : E + (h0 + 2) * HD]
            )
            .astype(np.float16)
            .reshape(ECH, 128, 128),
            "wv": np.ascontiguousarray(
                w_qkv[:, 2 * E + h0 * HD : 2 * E + (h0 + 2) * HD]
            )
            .astype(np.float16)
            .reshape(ECH, 128, 128),
            "bq": b_qkv[h0 * HD : (h0 + 2) * HD].astype(np.float16).reshape(1, 128),
            "bk": b_qkv[E + h0 * HD : E + (h0 + 2) * HD]
            .astype(np.float16)
            .reshape(1, 128),
            "bv": b_qkv[2 * E + h0 * HD : 2 * E + (h0 + 2) * HD]
            .astype(np.float16)
            .reshape(1, 128),
            "escB": np.ascontiguousarray(escB),
            "msB": np.ascontiguousarray(msB),
            "negmeB": np.ascontiguousarray(negmeB),
            "iotaKp": iotaKp,
            "corrT": np.stack([C[bh].T for bh in bhs], axis=1).astype(
                np.float16
            ),  # (65, 2, 2048)
            "wout2": np.concatenate(
                [w_out[(h0 + hh) * HD : (h0 + hh + 1) * HD] for hh in range(2)],
                axis=0,
            ).astype(np.float16),  # (128, 512)
            "reps": np.array([[reps]], np.int32),
        }
        in_maps.append(m)
    return in_maps


# ---------------------------------------------------------------- bass build
def _split_drain_waits_json(bir: bytes) -> bytes:
    """Workaround: this walrus build rejects instructions carrying more
    than one sync-wait command. Hoist excess waits onto inserted
    single-wait NoOps on the same engine directly before the
    instruction (same-engine program order makes this equivalent)."""
    import json

    m = json.loads(bir)
    limit = 1
    for f in m["functions"]:
        for bb in f["blocks"]:
            out = []
            for inst in bb["instructions"]:
                si = inst.get("sync_info")
                waits = (si.get("on_wait") or []) if si else []
                if len(waits) > limit:
                    for j, w in enumerate(waits[:-limit]):
                        pre = {
                            "engine": inst["engine"],
                            "ins": [],
                            "outs": [],
                            "name": f"{inst['name']}_w{j}",
                            "opcode": "NoOp",
                            "sync_info": {"on_update": [], "on_wait": [w]},
                        }
                        if "debug" in inst:
                            pre["debug"] = inst["debug"]
                        out.append(pre)
                    si["on_wait"] = waits[-limit:]
                out.append(inst)
            bb["instructions"] = out
    return json.dumps(m).encode()


def _build_nc(loop=True):
    import contextlib

    import concourse.bass as bass
    import concourse.tile as tile
    from concourse import mybir

    f16, f32, i32 = mybir.dt.float16, mybir.dt.float32, mybir.dt.int32
    nc = bass.Bass()
    d_xT = nc.dram_tensor("xT", (ECH, 128, T), f16, kind="ExternalInput")
    d_wq = nc.dram_tensor("wq", (ECH, 128, 128), f16, kind="ExternalInput")
    d_wk = nc.dram_tensor("wk", (ECH, 128, 128), f16, kind="ExternalInput")
    d_wv = nc.dram_tensor("wv", (ECH, 128, 128), f16, kind="ExternalInput")
    d_bq = nc.dram_tensor("bq", (1, 128), f16, kind="ExternalInput")
    d_bk = nc.dram_tensor("bk", (1, 128), f16, kind="ExternalInput")
    d_bv = nc.dram_tensor("bv", (1, 128), f16, kind="ExternalInput")
    d_escB = nc.dram_tensor("escB", (128, T), f16, kind="ExternalInput")
    d_msB = nc.dram_tensor("msB", (128, 2, T), f16, kind="ExternalInput")
    d_negmeB = nc.dram_tensor("negmeB", (128, 2, T), f16, kind="ExternalInput")
    d_iotaKp = nc.dram_tensor("iotaKp", (128, 2, KT_TILES), f32, kind="ExternalInput")
    d_corr = nc.dram_tensor("corrT", (HD + 1, 2, T), f16, kind="ExternalInput")
    d_wout2 = nc.dram_tensor("wout2", (128, 512), f16, kind="ExternalInput")
    d_reps = nc.dram_tensor("reps", (1, 1), i32, kind="ExternalInput")
    d_y = nc.dram_tensor("y", (T, E), f32, kind="ExternalOutput")

    with tile.TileContext(nc) as tc, contextlib.ExitStack() as stk:
        consts = stk.enter_context(tc.tile_pool(name="consts", bufs=2))
        qk = stk.enter_context(tc.tile_pool(name="qk", bufs=1))
        epool = stk.enter_context(tc.tile_pool(name="epool", bufs=3))
        small = stk.enter_context(tc.tile_pool(name="small", bufs=1))
        spool = stk.enter_context(tc.tile_pool(name="spool", bufs=2, space="PSUM"))
        pvpool = stk.enter_context(
            tc.tile_pool(name="pvpool", bufs=1, space="PSUM")
        )

        reps_sb = consts.tile([1, 1], i32)
        nc.sync.dma_start(reps_sb[:], d_reps[:])
        if loop:
            reps_val = nc.values_load(
                reps_sb[:],
                min_val=1,
                max_val=100000,
                skip_runtime_bounds_check=True,
            )
            loop_cm = tc.For_i(0, reps_val)
        else:
            import contextlib as _cl

            loop_cm = _cl.nullcontext()
        with loop_cm:
            # ---- load inputs
            xT = consts.tile([128, ECH, T], f16)
            nc.sync.dma_start(xT[:], d_xT.rearrange("c p t -> p c t"))
            wq = consts.tile([128, ECH, 128], f16)
            wk = consts.tile([128, ECH, 128], f16)
            wv = consts.tile([128, ECH, 128], f16)
            nc.sync.dma_start(wq[:], d_wq.rearrange("c p m -> p c m"))
            nc.sync.dma_start(wk[:], d_wk.rearrange("c p m -> p c m"))
            nc.sync.dma_start(wv[:], d_wv.rearrange("c p m -> p c m"))
            bq = consts.tile([1, 128], f16)
            bk = consts.tile([1, 128], f16)
            bv = consts.tile([1, 128], f16)
            nc.sync.dma_start(bq[:], d_bq[:])
            nc.sync.dma_start(bk[:], d_bk[:])
            nc.sync.dma_start(bv[:], d_bv[:])
            escB = consts.tile([128, T], f16)
            nc.sync.dma_start(escB[:], d_escB[:])
            msB = consts.tile([128, 2, T], f16)
            negmeB = consts.tile([128, 2, T], f16)
            nc.scalar.dma_start(msB[:], d_msB[:])
            nc.scalar.dma_start(negmeB[:], d_negmeB[:])
            iotaKp = consts.tile([128, 2, KT_TILES], f32)
            nc.sync.dma_start(iotaKp[:], d_iotaKp[:])
            corrT = consts.tile([HD + 1, 2, T], f16)
            nc.sync.dma_start(corrT[:], d_corr[:])
            wout2 = consts.tile([128, 512], f16)
            nc.sync.dma_start(wout2[:], d_wout2[:])
            ones_row = consts.tile([1, 512], f16)
            nc.vector.memset(ones_row[:], 1.0)
            ones32 = consts.tile([1, HD], f32)
            nc.vector.memset(ones32[:], 1.0)

            # ---- QKV projection (both heads at once; chan-major Q^T/K^T)
            # QT holds esc-prescaled Q^T so S^T = esc_q * (Q K^T)[q,k].
            QT = qk.tile([128, T], f16)
            KT = qk.tile([128, T], f16)
            for t4 in range(T // 512):
                sl = slice(t4 * 512, (t4 + 1) * 512)
                psq = spool.tile([128, 512], f32, tag="s")
                for ec in range(ECH):
                    nc.tensor.matmul(
                        psq[:], wq[:, ec, :], xT[:, ec, sl],
                        start=(ec == 0), stop=False,
                    )
                nc.tensor.matmul(psq[:], bq[:], ones_row[:], start=False, stop=True)
                nc.vector.tensor_mul(QT[:, sl], psq[:], escB[:, sl])
                psk = spool.tile([128, 512], f32, tag="s")
                for ec in range(ECH):
                    nc.tensor.matmul(
                        psk[:], wk[:, ec, :], xT[:, ec, sl],
                        start=(ec == 0), stop=False,
                    )
                nc.tensor.matmul(psk[:], bk[:], ones_row[:], start=False, stop=True)
                nc.vector.tensor_copy(KT[:, sl], psk[:])
            # V token-major, ones-augmented: vaug[:, kt, 0:65 | 65:130]
            vaug = qk.tile([128, KT_TILES, 2 * (HD + 1)], f16)
            nc.vector.memset(vaug[:, :, HD : HD + 1], 1.0)
            nc.vector.memset(vaug[:, :, 2 * HD + 1 : 2 * HD + 2], 1.0)
            for kt in range(KT_TILES):
                psv = spool.tile([128, 512], f32, tag="s")
                tsl = slice(kt * 128, (kt + 1) * 128)
                for ec in range(ECH):
                    nc.tensor.matmul(
                        psv[:, 0:128], xT[:, ec, tsl], wv[:, ec, :],
                        start=(ec == 0), stop=False,
                    )
                nc.tensor.matmul(
                    psv[:, 0:128], ones_row[:, 0:128], bv[:],
                    start=False, stop=True,
                )
                nc.vector.tensor_copy(vaug[:, kt, 0:HD], psv[:, 0:HD])
                nc.vector.tensor_copy(
                    vaug[:, kt, HD + 1 : 2 * HD + 1], psv[:, HD : 2 * HD]
                )

            # ---- attention per head: key-major S^T -> exp -> mask -> PV
            onorm2 = qk.tile([128, T], f16)  # both heads' normalized PV
            for hh in range(2):
                hsl = slice(hh * HD, (hh + 1) * HD)
                vsl = slice(hh * (HD + 1), (hh + 1) * (HD + 1))
                pv = pvpool.tile([HD + 1, T], f32)
                pending = None  # (E tile, kt) awaiting PV emission
                for kt in range(KT_TILES):
                    ksl = slice(kt * 128, (kt + 1) * 128)
                    Ekt = epool.tile([128, T], f16, tag="e")
                    for qh in range(T // 1024):
                        hq = slice(qh * 1024, (qh + 1) * 1024)
                        sps = spool.tile([128, 1024], f32, tag="s")
                        for qc in range(2):
                            qsl = slice(qh * 1024 + qc * 512, qh * 1024 + (qc + 1) * 512)
                            nc.tensor.matmul(
                                sps[:, qc * 512 : (qc + 1) * 512],
                                KT[hsl, ksl], QT[hsl, qsl],
                                start=True, stop=True,
                            )
                        nc.scalar.activation(
                            Ekt[:, hq], sps[:],
                            mybir.ActivationFunctionType.Exp,
                        )
                    mt = epool.tile([128, T], f16, tag="m1", name="mt")
                    nc.vector.tensor_scalar(
                        mt[:], msB[:, hh, :],
                        iotaKp[:, 0, kt : kt + 1], None,
                        mybir.AluOpType.is_le,
                    )
                    mt2 = epool.tile([128, T], f16, tag="m2", name="mt2")
                    nc.vector.tensor_scalar(
                        mt2[:], negmeB[:, hh, :],
                        iotaKp[:, 1, kt : kt + 1], None,
                        mybir.AluOpType.is_lt,
                    )
                    nc.vector.tensor_mul(Ekt[:], Ekt[:], mt[:])
                    nc.vector.tensor_mul(Ekt[:], Ekt[:], mt2[:])
                    if pending is not None:
                        pE, pkt = pending
                        for qc in range(T // 512):
                            qsl = slice(qc * 512, (qc + 1) * 512)
                            nc.tensor.matmul(
                                pv[:, qsl], vaug[:, pkt, vsl], pE[:, qsl],
                                start=(pkt == 0), stop=False,
                            )
                    pending = (Ekt, kt)
                pE, pkt = pending
                for qc in range(T // 512):
                    qsl = slice(qc * 512, (qc + 1) * 512)
                    nc.tensor.matmul(
                        pv[:, qsl], vaug[:, pkt, vsl], pE[:, qsl],
                        start=False, stop=True,
                    )
                # evacuate + host correction + normalize
                oaug = small.tile([HD + 1, T], f32, tag="oaug")
                nc.vector.tensor_add(oaug[:], pv[:], corrT[:, hh, :])
                sums = small.tile([1, T], f32, tag="sums")
                nc.sync.dma_start(sums[:], oaug[HD : HD + 1, :])
                recip = small.tile([1, T], f32, tag="recip")
                nc.vector.reciprocal_approx_fast(recip[:], sums[:])
                for qc in range(T // 512):
                    qsl = slice(qc * 512, (qc + 1) * 512)
                    bc = spool.tile([128, 512], f32, tag="s")
                    nc.tensor.matmul(
                        bc[0:HD, :], ones32[:], recip[:, qsl],
                        start=True, stop=True,
                    )
                    nc.vector.tensor_mul(
                        onorm2[hsl, qsl], oaug[0:HD, qsl], bc[0:HD, :]
                    )
            # ---- output projection (both heads in one K=128 contraction)
            for tt in range(T // 128):
                tsl = slice(tt * 128, (tt + 1) * 128)
                yp = spool.tile([128, 512], f32, tag="s")
                nc.tensor.matmul(
                    yp[:], onorm2[:, tsl], wout2[:], start=True, stop=True
                )
                y_sb = epool.tile([128, 512], f32, tag="ysb", name="y_sb")
                nc.scalar.copy(y_sb[:], yp[:])
                nc.sync.dma_start(d_y[tsl, :], y_sb[:])
    return nc


# ---------------------------------------------------------------- entry
def _get_nc():
    if "nc" not in _cache:
        from concourse import mybir

        nc = _build_nc()
        mybir.codegen_inst_isa_subclasses(nc)
        fixed = _split_drain_waits_json(nc.to_json_bytes())
        nc.to_json_bytes = lambda: fixed
        _cache["nc"] = nc
    return _cache["nc"]


def run_cores(inputs, reps=1):
    """Compile (cached) + run on 8 cores; returns list of per-core y."""
    from concourse.bass_utils import run_bass_kernel_spmd

    nc = _get_nc()
    in_maps = _prep_core_inputs(inputs, reps)
    res = run_bass_kernel_spmd(nc, in_maps, core_ids=list(range(NCORES)))
    return [r["y"] for r in res.results]


def kernel(**inputs):
    reps = int(os.environ.get("BASS_KERNEL_REPS", "1"))
    ys = run_cores(inputs, reps=reps)
    b_out = np.asarray(inputs["b_out"], np.float32)
    y = np.zeros((B, T, E), np.float32)
    for c in range(NCORES):
        y[c // 4] += ys[c]
    y += b_out[None, None, :]
    return y.astype(np.float32)
